# revision 57
# baseline (speedup 1.0000x reference)
import sys

sys.path.insert(0, "/opt/trn_rl_repo")

import numpy as np

P = 128          # partitions / tile edge
D = 128          # model dim
H = 4            # heads
DH = 32          # head dim
NCORES = 8

# Full-problem geometry (N=100000, E=800000). Each core owns NBLK node
# blocks of 128 nodes. Blocks are permuted into slots sorted by edge
# count (descending) so the per-slot tile count T_slot can be matched
# across cores; every slot's incident-edge list is padded to
# T_slot tiles of 128 edges so the SPMD program is uniform across cores.
NBLK_FULL = 98                      # 98*128 = 12544 own nodes/core
NPAD_FULL = NCORES * NBLK_FULL * P  # 100352 padded nodes


def _channel_perm():
    # torch reshape (N, DH, H): flat channel c = d*H + h. We relayout to
    # h-major c' = h*DH + d by permuting weight rows: perm[c'] = d*H + h.
    cp = np.arange(D)
    return (cp % DH) * H + (cp // DH)


def _build_program(NOWN, NBLK, T_slots):
    import concourse.bass as bass
    import concourse.tile as tile
    from concourse import bacc, mybir
    from concourse.masks import make_identity
    from contextlib import ExitStack

    dt = mybir.dt
    f32, f16, bf16, i32 = dt.float32, dt.float16, dt.bfloat16, dt.int32
    NT = int(sum(T_slots))    # edge tiles per core
    EPC = NT * P              # padded edges per core
    QT = NOWN // P            # x tiles for q projection (own nodes) == NBLK
    off_t = np.concatenate([[0], np.cumsum(T_slots)]).astype(int)

    nc = bacc.Bacc("TRN2", target_bir_lowering=False, debug=False,
                   num_devices=NCORES)

    # All inputs ship host-transposed (channel-major) so the contraction
    # dim is already on partitions. xct is x pre-gathered per edge:
    # xct[:, e] = x[col[e], :].
    xot_d = nc.dram_tensor("xot", [D, NOWN], f16, kind="ExternalInput").ap()
    xct_d = nc.dram_tensor("xct", [D, EPC], f16, kind="ExternalInput").ap()
    selt_d = nc.dram_tensor("selt", [P, EPC], f16, kind="ExternalInput").ap()
    sel_d = nc.dram_tensor("seld", [P, EPC], bf16, kind="ExternalInput").ap()
    wkv_d = nc.dram_tensor("wkv", [D, 2 * D], f16, kind="ExternalInput").ap()
    wq_d = nc.dram_tensor("wq", [D, D], f16, kind="ExternalInput").ap()
    wo_d = nc.dram_tensor("wo", [D, D], f16, kind="ExternalInput").ap()
    bq_d = nc.dram_tensor("bq", [1, D], f16, kind="ExternalInput").ap()
    bo_d = nc.dram_tensor("bo", [P, 1], f32, kind="ExternalInput").ap()

    # output is channel-major; host transposes back
    out_d = nc.dram_tensor("out", [D, NOWN], f32, kind="ExternalOutput").ap()

    AF = mybir.ActivationFunctionType
    OP = mybir.AluOpType

    with tile.TileContext(nc) as tc, ExitStack() as ctx:
        res = ctx.enter_context(tc.tile_pool(name="res", bufs=1))
        wkv_sb = res.tile([D, 2 * D], f16, name="wkv_sb")
        wq_sb = res.tile([D, D], f16, name="wq_sb")
        wo_sb = res.tile([D, D], f16, name="wo_sb")
        bq_sb = res.tile([1, D], f16, name="bq_sb")
        bo_sb = res.tile([P, 1], f32, name="bo_sb")
        ones_sb = res.tile([1, P], f16, name="ones_sb")
        ident = res.tile([P, P], f16, name="ident")
        q_sb = res.tile([P, QT, D], f16, name="q_sb")  # resident q, all own nodes

        for sb_t, dr_t in [(wkv_sb, wkv_d), (wq_sb, wq_d), (wo_sb, wo_d),
                           (bq_sb, bq_d), (bo_sb, bo_d)]:
            nc.sync.dma_start(sb_t[:], dr_t[:])
        nc.vector.memset(ones_sb[:], 1.0)
        make_identity(nc, ident[:])

        # ---- phase A: q projection for own nodes, kept in SBUF ----
        CH = 4  # x tiles per DMA chunk
        with tc.tile_pool(name="xa", bufs=3) as xa, \
             tc.tile_pool(name="pa", bufs=2, space="PSUM") as pa:
            for j0 in range(0, QT, CH):
                c = min(CH, QT - j0)
                xo16 = xa.tile([P, c * P], f16, name="xo16")
                nc.sync.dma_start(xo16[:], xot_d[:, j0 * P:(j0 + c) * P])
                q_ps = pa.tile([P, CH, D], f32, name="q_ps")
                for t in range(c):
                    nc.tensor.matmul(q_ps[:, t, :], lhsT=ones_sb[:],
                                     rhs=bq_sb[:], start=True, stop=False)
                    nc.tensor.matmul(q_ps[:, t, :],
                                     lhsT=xo16[:, t * P:(t + 1) * P],
                                     rhs=wq_sb[:], start=False, stop=True)
                nc.scalar.copy(q_sb[:, j0:j0 + c, :], q_ps[:, 0:c, :])

        # ---- phase B: per-slot edge-streaming, tails grouped by G ----
        G = 2
        def emit_tail(yb, b0, gn):
            # transposed output projection for a finished group (deferred
            # one group so the PE never waits on the normalization)
            yT_g = ep.tile([P, G, D], f16, name="yT_g")
            o_g = ep.tile([P, G, D], f32, name="o_g")
            for g in range(gn):
                nc.tensor.transpose(yT_g[:, g, :], yb[:, g, :], ident[:])
            yTs = eg.tile([P, G, D], f16, name="yTs")
            nc.vector.tensor_copy(yTs[:, 0:gn, :], yT_g[:, 0:gn, :])
            # wo stays stationary in the PE; bias folds into the ACT
            # evacuation (per-partition).
            for g in range(gn):
                nc.tensor.matmul(o_g[:, g, :], lhsT=wo_sb[:],
                                 rhs=yTs[:, g, :], start=True, stop=True)
            o_sb = eg.tile([P, G, D], f32, name="o_sb")
            nc.scalar.activation(o_sb[:, 0:gn, :], o_g[:, 0:gn, :],
                                 AF.Identity, bias=bo_sb[:])
            nc.sync.dma_start(
                out_d[:, b0 * P:(b0 + gn) * P],
                o_sb[:, 0:gn, :].rearrange("p g i -> p (g i)"))

        with tc.tile_pool(name="eg", bufs=4) as eg, \
             tc.tile_pool(name="kp", bufs=2, space="PSUM") as kp, \
             tc.tile_pool(name="qx", bufs=1, space="PSUM") as qx, \
             tc.tile_pool(name="yp", bufs=1, space="PSUM") as yp, \
             tc.tile_pool(name="ep", bufs=1, space="PSUM") as ep:
            pend = None
            for b0 in range(0, NBLK, G):
                gn = min(G, NBLK - b0)
                Tg = int(sum(T_slots[b0:b0 + gn]))
                E0g = off_t[b0] * P
                # grouped loads: one DMA per input stream per group
                xc_g = eg.tile([P, Tg * P], f16, name="xc_g")
                nc.sync.dma_start(xc_g[:], xct_d[:, E0g:E0g + Tg * P])
                selt_g = eg.tile([P, Tg * P], f16, name="selt_g")
                nc.sync.dma_start(selt_g[:], selt_d[:, E0g:E0g + Tg * P])
                # one-hot segment matrix for the segment-sum matmuls
                sel_g = eg.tile([P, Tg, P], bf16, name="sel_g")
                nc.sync.dma_start(
                    sel_g[:].rearrange("p t j -> p (t j)"),
                    sel_d[:, E0g:E0g + Tg * P])
                # ypre rows padded to 256 f32 so each slot's segment-sum
                # stays within one PSUM bank
                ypre_g = yp.tile([P, G, 256], f32, name="ypre_g")
                # pass 1: projections + scores for all slots of the group,
                # so the PE never idles waiting for a slot's wext
                wexts = []
                for g in range(gn):
                    b = b0 + g
                    T = int(T_slots[b])
                    toff = off_t[b] - off_t[b0]

                    # per-edge k|v projection and q broadcast
                    kvt = eg.tile([P, T, 2 * D], f16, name="kvt")
                    qx_ps = qx.tile([P, T, D], f32, name="qx_ps")
                    for t in range(T):
                        tg = toff + t
                        kv_ps = kp.tile([P, 2 * D], f32, name="kv_ps")
                        nc.tensor.matmul(kv_ps[:],
                                         lhsT=xc_g[:, tg * P:(tg + 1) * P],
                                         rhs=wkv_sb[:], start=True, stop=True)
                        nc.tensor.matmul(qx_ps[:, t, :],
                                         lhsT=selt_g[:, tg * P:(tg + 1) * P],
                                         rhs=q_sb[:, b, :],
                                         start=True, stop=True)
                        if t % 4 == 0:
                            nc.vector.tensor_copy(kvt[:, t, :], kv_ps[:])
                        else:
                            nc.scalar.copy(kvt[:, t, :], kv_ps[:])

                    # evacuate qx on the scalar engine so the score multiply
                    # runs as an all-SBUF 16-bit op on the vector engine
                    qx16 = eg.tile([P, T, D], f16, name="qx16")
                    nc.scalar.copy(qx16[:], qx_ps[:])
                    # scores: s[e,h] = sum_d qx[e,hd]*k[e,hd]
                    prod = eg.tile([P, T, D], f16, name="prod")
                    nc.vector.tensor_tensor(out=prod[:], in0=qx16[:],
                                            in1=kvt[:, :, 0:D], op=OP.mult)
                    s_b = eg.tile([P, T, H], f16, name="s_b")
                    with nc.allow_low_precision("scores bounded by ~16; "
                                                "f16 ulp 0.008 is ample"):
                        nc.vector.tensor_reduce(
                            out=s_b[:],
                            in_=prod[:].rearrange("p t (h d) -> p t h d", h=H),
                            axis=mybir.AxisListType.X, op=OP.add)
                    wext = eg.tile([P, T, D + H], bf16, name="wext")
                    nc.scalar.activation(wext[:, :, D:D + H], s_b[:], AF.Exp)
                    nc.vector.tensor_tensor(
                        out=wext[:, :, 0:D].rearrange(
                            "p t (h d) -> p t h d", h=H),
                        in0=kvt[:, :, D:2 * D].rearrange(
                            "p t (h d) -> p t h d", h=H),
                        in1=wext[:, :, D:D + H].to_broadcast((P, T, H, DH)),
                        op=OP.mult)
                    wexts.append((T, toff, wext))

                # pass 2: segment-sums over incoming edges (denominator
                # rides along in columns D:D+H)
                for g in range(gn):
                    T, toff, wext = wexts[g]
                    for t in range(T):
                        nc.tensor.matmul(ypre_g[:, g, 0:D + H],
                                         lhsT=sel_g[:, toff + t, :],
                                         rhs=wext[:, t, :],
                                         start=(t == 0), stop=(t == T - 1))

                # inline normalization (frees ypre_g for the next group)
                zr = eg.tile([P, G, H], f32, name="zr")
                nc.vector.tensor_scalar_add(zr[:, 0:gn, :],
                                            ypre_g[:, 0:gn, D:D + H], 1e-30)
                rz = eg.tile([P, G, H], f32, name="rz")
                nc.vector.reciprocal(rz[:, 0:gn, :], zr[:, 0:gn, :])
                yb = eg.tile([P, G, D], f16, name="yb")
                nc.vector.tensor_tensor(
                    out=yb[:, 0:gn, :].rearrange("p g (h d) -> p g h d", h=H),
                    in0=ypre_g[:, 0:gn, 0:D].rearrange(
                        "p g (h d) -> p g h d", h=H),
                    in1=rz[:, 0:gn, :].to_broadcast((P, gn, H, DH)),
                    op=OP.mult)
                if pend is not None:
                    emit_tail(*pend)
                pend = (yb, b0, gn)
            emit_tail(*pend)

    nc.compile()
    return nc


def _edge_layout(row, NOWN, NBLK):
    """Per-core slot permutation and matched per-slot tile counts."""
    row = np.asarray(row, np.int64)
    orders, cnts_all = [], []
    for c in range(NCORES):
        lo, hi = c * NOWN, (c + 1) * NOWN
        e0 = np.searchsorted(row, lo, "left")
        e1 = np.searchsorted(row, hi, "left")
        blk = (row[e0:e1] - lo) // P
        cnts = np.bincount(blk, minlength=NBLK)
        order = np.argsort(-cnts, kind="stable")
        orders.append(order)
        cnts_all.append(cnts[order])
    ranked = np.stack(cnts_all)              # [NCORES, NBLK] descending
    T_slots = np.maximum(1, -(-ranked.max(0) // P))  # ceil
    return orders, T_slots


def _prepare_inputs(x, row, col, Wq, bq, Wk, bk, Wv, bv, Wo, bo,
                    NOWN, NBLK, orders, T_slots):
    """Host-side sharding: per-core padded edge lists + permuted weights.

    bk drops out of the softmax (constant per destination row); bv folds
    through the output projection exactly (sum_e a_e = 1)."""
    N = x.shape[0]
    perm = _channel_perm()
    s = np.sqrt(float(H))
    wkv_in = np.ascontiguousarray(
        np.concatenate([Wk[perm, :].T, Wv[perm, :].T], axis=1)
    ).astype(np.float16)
    wq_in = np.ascontiguousarray((Wq[perm, :] / s).T).astype(np.float16)
    wo_in = np.ascontiguousarray(Wo[:, perm].T).astype(np.float16)
    bq_in = (bq[perm] / s).reshape(1, D).astype(np.float16)
    bo_in = (bo + Wo @ bv).reshape(P, 1).astype(np.float32)

    NPAD = NCORES * NOWN
    x_pad = np.zeros((NPAD, D), np.float32)
    x_pad[:N] = x
    x16 = x_pad.astype(np.float16)

    NT = int(T_slots.sum())
    EPC = NT * P
    off_e = np.concatenate([[0], np.cumsum(T_slots * P)]).astype(np.int64)
    in_maps = []
    for c in range(NCORES):
        lo, hi = c * NOWN, (c + 1) * NOWN
        e0 = np.searchsorted(row, lo, "left")
        e1 = np.searchsorted(row, hi, "left")
        rows_c = (row[e0:e1] - lo).astype(np.int64)
        cols_c = col[e0:e1].astype(np.int64)
        blk = rows_c // P
        blk_starts = np.searchsorted(blk, np.arange(NBLK), "left")
        rank = np.arange(rows_c.shape[0]) - blk_starts[blk]
        order = orders[c]
        slot_of = np.empty(NBLK, np.int64)
        slot_of[order] = np.arange(NBLK)
        pos = off_e[slot_of[blk]] + rank
        rl_local = rows_c % P
        ci = np.zeros(EPC, np.int64)
        ci[pos] = cols_c
        # pre-gathered x[col[e]] per edge, channel-major
        xct = np.ascontiguousarray(x16[ci].T)
        # one-hot selection matrices: selt[j, e] (q broadcast) and its
        # per-tile transpose sel[e%128, tile*128 + j] (segment sum)
        import ml_dtypes
        selt = np.zeros((P, EPC), np.float16)
        selt[rl_local, pos] = 1.0
        seld = np.zeros((P, EPC), ml_dtypes.bfloat16)
        seld[pos % P, (pos // P) * P + rl_local] = 1.0
        # own x in slot order
        xo = x16[lo:hi].reshape(NBLK, P, D)[order].reshape(NOWN, D)
        in_maps.append({
            "xot": np.ascontiguousarray(xo.T),
            "xct": xct,
            "wkv": wkv_in, "wq": wq_in, "wo": wo_in,
            "bq": bq_in, "bo": bo_in,
            "selt": selt, "seld": seld,
        })
    return in_maps


def _install_ntff_hook():
    """The agent image's antenv lacks axon_hooks; inject it so trace=True
    can drive NTFF profiling through libaxon_pjrt.so."""
    import importlib
    try:
        importlib.import_module("antenv.axon_hooks")
        return
    except ImportError:
        pass
    import types
    if "/root/.axon_site" not in sys.path:
        sys.path.insert(0, "/root/.axon_site")
    from trn_agent_boot.trn_boot import _ntff_profile_via_ctypes
    hook = _ntff_profile_via_ctypes("/opt/axon/libaxon_pjrt.so")
    mod = types.ModuleType("antenv.axon_hooks")
    state = {"hook": hook}
    mod.get_axon_ntff_profile_hook = lambda: state["hook"]
    mod.set_axon_ntff_profile_hook = lambda h: state.update(hook=h)
    import antenv
    antenv.axon_hooks = mod
    sys.modules["antenv.axon_hooks"] = mod


def _enable_ldw_opt():
    """Flip walrus --enable-ldw-opt on: our stream is LDWEIGHTS-bound and
    all matmul operands are 16-bit (the flag's f32 codegen hazard doesn't
    apply)."""
    from concourse import bass_utils
    if getattr(bass_utils, "_ldw_patched", False):
        return
    orig = bass_utils.run_command

    def patched(cmd, *a, **kw):
        cmd = [c.replace("--enable-ldw-opt=false", "--enable-ldw-opt=true")
               if isinstance(c, str) else c for c in cmd]
        return orig(cmd, *a, **kw)

    bass_utils.run_command = patched
    bass_utils._ldw_patched = True


def run(x, row, col, Wq, bq, Wk, bk, Wv, bv, Wo, bo, NBLK=NBLK_FULL,
        trace=False, tmpdir=None):
    import os
    from concourse import bass_utils
    from concourse.bass_utils import run_bass_kernel_spmd
    if os.environ.get("KERNEL_LDW_OPT", "0") == "1":
        _enable_ldw_opt()
    if trace:
        _install_ntff_hook()
        bass_utils.upload_artifacts = lambda d: "local://" + d

    x = np.asarray(x, np.float32)
    row = np.asarray(row, np.int64)
    col = np.asarray(col, np.int64)
    N = x.shape[0]
    NOWN = NBLK * P
    NPAD = NCORES * NOWN
    assert NPAD >= N
    orders, T_slots = _edge_layout(row, NOWN, NBLK)
    nc = _build_program(NOWN, NBLK, T_slots)
    in_maps = _prepare_inputs(
        x, row, col,
        np.asarray(Wq, np.float32), np.asarray(bq, np.float32),
        np.asarray(Wk, np.float32), np.asarray(bk, np.float32),
        np.asarray(Wv, np.float32), np.asarray(bv, np.float32),
        np.asarray(Wo, np.float32), np.asarray(bo, np.float32),
        NOWN, NBLK, orders, T_slots)
    res = run_bass_kernel_spmd(nc, in_maps, list(range(NCORES)), trace=trace,
                               tmpdir=tmpdir)
    out = np.empty((NPAD, D), np.float32)
    for c in range(NCORES):
        lo = c * NOWN
        res_t = res.results[c]["out"].T            # [NOWN, D], slot order
        perm_rows = (orders[c][:, None] * P + np.arange(P)).ravel()
        out[lo + perm_rows] = res_t
    return out[:N], res


def kernel(**inputs):
    out, _ = run(**inputs)
    return out


# revision 58
# speedup vs baseline: 1.5367x; 1.5367x over previous
import sys

sys.path.insert(0, "/opt/trn_rl_repo")

import numpy as np

P = 128          # partitions / tile edge
D = 128          # model dim
H = 4            # heads
DH = 32          # head dim
NCORES = 8

# Full-problem geometry (N=100000, E=800000). Each core owns NBLK node
# blocks of 128 nodes. Blocks are permuted into slots sorted by edge
# count (descending) so the per-slot tile count T_slot can be matched
# across cores; every slot's incident-edge list is padded to
# T_slot tiles of 128 edges so the SPMD program is uniform across cores.
NBLK_FULL = 98                      # 98*128 = 12544 own nodes/core
NPAD_FULL = NCORES * NBLK_FULL * P  # 100352 padded nodes


def _channel_perm():
    # torch reshape (N, DH, H): flat channel c = d*H + h. We relayout to
    # h-major c' = h*DH + d by permuting weight rows: perm[c'] = d*H + h.
    cp = np.arange(D)
    return (cp % DH) * H + (cp // DH)


def _build_program(NOWN, NBLK, T_slots):
    import concourse.bass as bass
    import concourse.tile as tile
    from concourse import bacc, mybir
    from concourse.masks import make_identity
    from contextlib import ExitStack

    dt = mybir.dt
    f32, f16, bf16, i32 = dt.float32, dt.float16, dt.bfloat16, dt.int32
    NT = int(sum(T_slots))    # edge tiles per core
    EPC = NT * P              # padded edges per core
    QT = NOWN // P            # x tiles for q projection (own nodes) == NBLK
    off_t = np.concatenate([[0], np.cumsum(T_slots)]).astype(int)

    nc = bacc.Bacc("TRN2", target_bir_lowering=False, debug=False,
                   num_devices=NCORES)

    # All inputs ship host-transposed (channel-major) so the contraction
    # dim is already on partitions. xct is x pre-gathered per edge:
    # xct[:, e] = x[col[e], :].
    xot_d = nc.dram_tensor("xot", [D, NOWN], f16, kind="ExternalInput").ap()
    xct_d = nc.dram_tensor("xct", [D, EPC], f16, kind="ExternalInput").ap()
    selt_d = nc.dram_tensor("selt", [P, EPC], f16, kind="ExternalInput").ap()
    sel_d = nc.dram_tensor("seld", [P, EPC], bf16, kind="ExternalInput").ap()
    wkv_d = nc.dram_tensor("wkv", [D, 2 * D], f16, kind="ExternalInput").ap()
    wq_d = nc.dram_tensor("wq", [D, D], f16, kind="ExternalInput").ap()
    wo_d = nc.dram_tensor("wo", [D, D], f16, kind="ExternalInput").ap()
    bq_d = nc.dram_tensor("bq", [1, D], f16, kind="ExternalInput").ap()
    bo_d = nc.dram_tensor("bo", [P, 1], f32, kind="ExternalInput").ap()

    # output is channel-major; host transposes back
    out_d = nc.dram_tensor("out", [D, NOWN], f32, kind="ExternalOutput").ap()

    AF = mybir.ActivationFunctionType
    OP = mybir.AluOpType

    with tile.TileContext(nc) as tc, ExitStack() as ctx:
        res = ctx.enter_context(tc.tile_pool(name="res", bufs=1))
        wkv_sb = res.tile([D, 2 * D], f16, name="wkv_sb")
        wq_sb = res.tile([D, D], f16, name="wq_sb")
        wo_sb = res.tile([D, D], f16, name="wo_sb")
        bq_sb = res.tile([1, D], f16, name="bq_sb")
        bo_sb = res.tile([P, 1], f32, name="bo_sb")
        ones_sb = res.tile([1, P], f16, name="ones_sb")
        ident = res.tile([P, P], f16, name="ident")
        q_sb = res.tile([P, QT, D], f16, name="q_sb")  # resident q, all own nodes

        for sb_t, dr_t in [(wkv_sb, wkv_d), (wq_sb, wq_d), (wo_sb, wo_d),
                           (bq_sb, bq_d), (bo_sb, bo_d)]:
            nc.sync.dma_start(sb_t[:], dr_t[:])
        nc.vector.memset(ones_sb[:], 1.0)
        make_identity(nc, ident[:])

        # ---- phase A: q projection for own nodes, kept in SBUF ----
        CH = 4  # x tiles per DMA chunk
        with tc.tile_pool(name="xa", bufs=3) as xa, \
             tc.tile_pool(name="pa", bufs=2, space="PSUM") as pa:
            for j0 in range(0, QT, CH):
                c = min(CH, QT - j0)
                xo16 = xa.tile([P, c * P], f16, name="xo16")
                nc.sync.dma_start(xo16[:], xot_d[:, j0 * P:(j0 + c) * P])
                q_ps = pa.tile([P, CH, D], f32, name="q_ps")
                for t in range(c):
                    nc.tensor.matmul(q_ps[:, t, :], lhsT=ones_sb[:],
                                     rhs=bq_sb[:], start=True, stop=False)
                    nc.tensor.matmul(q_ps[:, t, :],
                                     lhsT=xo16[:, t * P:(t + 1) * P],
                                     rhs=wq_sb[:], start=False, stop=True)
                nc.scalar.copy(q_sb[:, j0:j0 + c, :], q_ps[:, 0:c, :])

        # ---- phase B: per-slot edge-streaming, tails grouped by G ----
        G = 2
        def emit_tail(yb, b0, gn):
            # transposed output projection for a finished group (deferred
            # one group so the PE never waits on the normalization)
            yT_g = ep.tile([P, G, D], f16, name="yT_g")
            o_g = ep.tile([P, G, D], f32, name="o_g")
            for g in range(gn):
                nc.tensor.transpose(yT_g[:, g, :], yb[:, g, :], ident[:])
            yTs = eg.tile([P, G, D], f16, name="yTs")
            nc.vector.tensor_copy(yTs[:, 0:gn, :], yT_g[:, 0:gn, :])
            # wo stays stationary in the PE; bias folds into the ACT
            # evacuation (per-partition).
            for g in range(gn):
                nc.tensor.matmul(o_g[:, g, :], lhsT=wo_sb[:],
                                 rhs=yTs[:, g, :], start=True, stop=True)
            o_sb = eg.tile([P, G, D], f32, name="o_sb")
            nc.scalar.activation(o_sb[:, 0:gn, :], o_g[:, 0:gn, :],
                                 AF.Identity, bias=bo_sb[:])
            nc.sync.dma_start(
                out_d[:, b0 * P:(b0 + gn) * P],
                o_sb[:, 0:gn, :].rearrange("p g i -> p (g i)"))

        with tc.tile_pool(name="eg", bufs=4) as eg, \
             tc.tile_pool(name="kp", bufs=2, space="PSUM") as kp, \
             tc.tile_pool(name="qx", bufs=1, space="PSUM") as qx, \
             tc.tile_pool(name="yp", bufs=1, space="PSUM") as yp, \
             tc.tile_pool(name="ep", bufs=1, space="PSUM") as ep:
            pend = None
            for b0 in range(0, NBLK, G):
                gn = min(G, NBLK - b0)
                Tg = int(sum(T_slots[b0:b0 + gn]))
                E0g = off_t[b0] * P
                # grouped loads: one DMA per input stream per group
                xc_g = eg.tile([P, Tg * P], f16, name="xc_g")
                nc.sync.dma_start(xc_g[:], xct_d[:, E0g:E0g + Tg * P])
                selt_g = eg.tile([P, Tg * P], f16, name="selt_g")
                nc.sync.dma_start(selt_g[:], selt_d[:, E0g:E0g + Tg * P])
                # one-hot segment matrix for the segment-sum matmuls
                sel_g = eg.tile([P, Tg, P], bf16, name="sel_g")
                nc.sync.dma_start(
                    sel_g[:].rearrange("p t j -> p (t j)"),
                    sel_d[:, E0g:E0g + Tg * P])
                # ypre rows padded to 256 f32 so each slot's segment-sum
                # stays within one PSUM bank
                ypre_g = yp.tile([P, G, 256], f32, name="ypre_g")
                # pass 1: projections + scores for all slots of the group,
                # so the PE never idles waiting for a slot's wext
                wexts = []
                for g in range(gn):
                    b = b0 + g
                    T = int(T_slots[b])
                    toff = off_t[b] - off_t[b0]

                    # per-edge k|v projection and q broadcast; kv lands in
                    # paired PSUM tiles (one bank) so each evacuation cast
                    # moves two tiles at once
                    kvt = eg.tile([P, T, 2 * D], f16, name="kvt")
                    qx_ps = qx.tile([P, T, D], f32, name="qx_ps")
                    for t0 in range(0, T, 2):
                        n2 = min(2, T - t0)
                        kv2 = kp.tile([P, 2, 2 * D], f32, name="kv2")
                        for dt2 in range(n2):
                            t = t0 + dt2
                            tg = toff + t
                            nc.tensor.matmul(kv2[:, dt2, :],
                                             lhsT=xc_g[:, tg * P:(tg + 1) * P],
                                             rhs=wkv_sb[:],
                                             start=True, stop=True)
                            nc.tensor.matmul(qx_ps[:, t, :],
                                             lhsT=selt_g[:, tg * P:(tg + 1) * P],
                                             rhs=q_sb[:, b, :],
                                             start=True, stop=True)
                        if (t0 // 2) % 4 == 0:
                            nc.vector.tensor_copy(kvt[:, t0:t0 + n2, :],
                                                  kv2[:, 0:n2, :])
                        else:
                            nc.scalar.copy(kvt[:, t0:t0 + n2, :],
                                           kv2[:, 0:n2, :])

                    # evacuate qx on the scalar engine so the score multiply
                    # runs as an all-SBUF 16-bit op on the vector engine
                    qx16 = eg.tile([P, T, D], f16, name="qx16")
                    nc.scalar.copy(qx16[:], qx_ps[:])
                    # scores: s[e,h] = sum_d qx[e,hd]*k[e,hd]
                    prod = eg.tile([P, T, D], f16, name="prod")
                    nc.vector.tensor_tensor(out=prod[:], in0=qx16[:],
                                            in1=kvt[:, :, 0:D], op=OP.mult)
                    s_b = eg.tile([P, T, H], f16, name="s_b")
                    with nc.allow_low_precision("scores bounded by ~16; "
                                                "f16 ulp 0.008 is ample"):
                        nc.vector.tensor_reduce(
                            out=s_b[:],
                            in_=prod[:].rearrange("p t (h d) -> p t h d", h=H),
                            axis=mybir.AxisListType.X, op=OP.add)
                    wext = eg.tile([P, T, D + H], bf16, name="wext")
                    nc.scalar.activation(wext[:, :, D:D + H], s_b[:], AF.Exp)
                    nc.vector.tensor_tensor(
                        out=wext[:, :, 0:D].rearrange(
                            "p t (h d) -> p t h d", h=H),
                        in0=kvt[:, :, D:2 * D].rearrange(
                            "p t (h d) -> p t h d", h=H),
                        in1=wext[:, :, D:D + H].to_broadcast((P, T, H, DH)),
                        op=OP.mult)
                    wexts.append((T, toff, wext))

                # pass 2: segment-sums over incoming edges (denominator
                # rides along in columns D:D+H)
                for g in range(gn):
                    T, toff, wext = wexts[g]
                    for t in range(T):
                        nc.tensor.matmul(ypre_g[:, g, 0:D + H],
                                         lhsT=sel_g[:, toff + t, :],
                                         rhs=wext[:, t, :],
                                         start=(t == 0), stop=(t == T - 1))

                # inline normalization (frees ypre_g for the next group)
                zr = eg.tile([P, G, H], f32, name="zr")
                nc.vector.tensor_scalar_add(zr[:, 0:gn, :],
                                            ypre_g[:, 0:gn, D:D + H], 1e-30)
                rz = eg.tile([P, G, H], f32, name="rz")
                nc.vector.reciprocal(rz[:, 0:gn, :], zr[:, 0:gn, :])
                yb = eg.tile([P, G, D], f16, name="yb")
                nc.vector.tensor_tensor(
                    out=yb[:, 0:gn, :].rearrange("p g (h d) -> p g h d", h=H),
                    in0=ypre_g[:, 0:gn, 0:D].rearrange(
                        "p g (h d) -> p g h d", h=H),
                    in1=rz[:, 0:gn, :].to_broadcast((P, gn, H, DH)),
                    op=OP.mult)
                if pend is not None:
                    emit_tail(*pend)
                pend = (yb, b0, gn)
            emit_tail(*pend)

    nc.compile()
    return nc


def _edge_layout(row, NOWN, NBLK):
    """Per-core slot permutation and matched per-slot tile counts."""
    row = np.asarray(row, np.int64)
    orders, cnts_all = [], []
    for c in range(NCORES):
        lo, hi = c * NOWN, (c + 1) * NOWN
        e0 = np.searchsorted(row, lo, "left")
        e1 = np.searchsorted(row, hi, "left")
        blk = (row[e0:e1] - lo) // P
        cnts = np.bincount(blk, minlength=NBLK)
        order = np.argsort(-cnts, kind="stable")
        orders.append(order)
        cnts_all.append(cnts[order])
    ranked = np.stack(cnts_all)              # [NCORES, NBLK] descending
    T_slots = np.maximum(1, -(-ranked.max(0) // P))  # ceil
    return orders, T_slots


def _prepare_inputs(x, row, col, Wq, bq, Wk, bk, Wv, bv, Wo, bo,
                    NOWN, NBLK, orders, T_slots):
    """Host-side sharding: per-core padded edge lists + permuted weights.

    bk drops out of the softmax (constant per destination row); bv folds
    through the output projection exactly (sum_e a_e = 1)."""
    N = x.shape[0]
    perm = _channel_perm()
    s = np.sqrt(float(H))
    wkv_in = np.ascontiguousarray(
        np.concatenate([Wk[perm, :].T, Wv[perm, :].T], axis=1)
    ).astype(np.float16)
    wq_in = np.ascontiguousarray((Wq[perm, :] / s).T).astype(np.float16)
    wo_in = np.ascontiguousarray(Wo[:, perm].T).astype(np.float16)
    bq_in = (bq[perm] / s).reshape(1, D).astype(np.float16)
    bo_in = (bo + Wo @ bv).reshape(P, 1).astype(np.float32)

    NPAD = NCORES * NOWN
    x_pad = np.zeros((NPAD, D), np.float32)
    x_pad[:N] = x
    x16 = x_pad.astype(np.float16)

    NT = int(T_slots.sum())
    EPC = NT * P
    off_e = np.concatenate([[0], np.cumsum(T_slots * P)]).astype(np.int64)
    in_maps = []
    for c in range(NCORES):
        lo, hi = c * NOWN, (c + 1) * NOWN
        e0 = np.searchsorted(row, lo, "left")
        e1 = np.searchsorted(row, hi, "left")
        rows_c = (row[e0:e1] - lo).astype(np.int64)
        cols_c = col[e0:e1].astype(np.int64)
        blk = rows_c // P
        blk_starts = np.searchsorted(blk, np.arange(NBLK), "left")
        rank = np.arange(rows_c.shape[0]) - blk_starts[blk]
        order = orders[c]
        slot_of = np.empty(NBLK, np.int64)
        slot_of[order] = np.arange(NBLK)
        pos = off_e[slot_of[blk]] + rank
        rl_local = rows_c % P
        ci = np.zeros(EPC, np.int64)
        ci[pos] = cols_c
        # pre-gathered x[col[e]] per edge, channel-major
        xct = np.ascontiguousarray(x16[ci].T)
        # one-hot selection matrices: selt[j, e] (q broadcast) and its
        # per-tile transpose sel[e%128, tile*128 + j] (segment sum)
        import ml_dtypes
        selt = np.zeros((P, EPC), np.float16)
        selt[rl_local, pos] = 1.0
        seld = np.zeros((P, EPC), ml_dtypes.bfloat16)
        seld[pos % P, (pos // P) * P + rl_local] = 1.0
        # own x in slot order
        xo = x16[lo:hi].reshape(NBLK, P, D)[order].reshape(NOWN, D)
        in_maps.append({
            "xot": np.ascontiguousarray(xo.T),
            "xct": xct,
            "wkv": wkv_in, "wq": wq_in, "wo": wo_in,
            "bq": bq_in, "bo": bo_in,
            "selt": selt, "seld": seld,
        })
    return in_maps


def _install_ntff_hook():
    """The agent image's antenv lacks axon_hooks; inject it so trace=True
    can drive NTFF profiling through libaxon_pjrt.so."""
    import importlib
    try:
        importlib.import_module("antenv.axon_hooks")
        return
    except ImportError:
        pass
    import types
    if "/root/.axon_site" not in sys.path:
        sys.path.insert(0, "/root/.axon_site")
    from trn_agent_boot.trn_boot import _ntff_profile_via_ctypes
    hook = _ntff_profile_via_ctypes("/opt/axon/libaxon_pjrt.so")
    mod = types.ModuleType("antenv.axon_hooks")
    state = {"hook": hook}
    mod.get_axon_ntff_profile_hook = lambda: state["hook"]
    mod.set_axon_ntff_profile_hook = lambda h: state.update(hook=h)
    import antenv
    antenv.axon_hooks = mod
    sys.modules["antenv.axon_hooks"] = mod


def _enable_ldw_opt():
    """Flip walrus --enable-ldw-opt on: our stream is LDWEIGHTS-bound and
    all matmul operands are 16-bit (the flag's f32 codegen hazard doesn't
    apply)."""
    from concourse import bass_utils
    if getattr(bass_utils, "_ldw_patched", False):
        return
    orig = bass_utils.run_command

    def patched(cmd, *a, **kw):
        cmd = [c.replace("--enable-ldw-opt=false", "--enable-ldw-opt=true")
               if isinstance(c, str) else c for c in cmd]
        return orig(cmd, *a, **kw)

    bass_utils.run_command = patched
    bass_utils._ldw_patched = True


def run(x, row, col, Wq, bq, Wk, bk, Wv, bv, Wo, bo, NBLK=NBLK_FULL,
        trace=False, tmpdir=None):
    import os
    from concourse import bass_utils
    from concourse.bass_utils import run_bass_kernel_spmd
    if os.environ.get("KERNEL_LDW_OPT", "0") == "1":
        _enable_ldw_opt()
    if trace:
        _install_ntff_hook()
        bass_utils.upload_artifacts = lambda d: "local://" + d

    x = np.asarray(x, np.float32)
    row = np.asarray(row, np.int64)
    col = np.asarray(col, np.int64)
    N = x.shape[0]
    NOWN = NBLK * P
    NPAD = NCORES * NOWN
    assert NPAD >= N
    orders, T_slots = _edge_layout(row, NOWN, NBLK)
    nc = _build_program(NOWN, NBLK, T_slots)
    in_maps = _prepare_inputs(
        x, row, col,
        np.asarray(Wq, np.float32), np.asarray(bq, np.float32),
        np.asarray(Wk, np.float32), np.asarray(bk, np.float32),
        np.asarray(Wv, np.float32), np.asarray(bv, np.float32),
        np.asarray(Wo, np.float32), np.asarray(bo, np.float32),
        NOWN, NBLK, orders, T_slots)
    res = run_bass_kernel_spmd(nc, in_maps, list(range(NCORES)), trace=trace,
                               tmpdir=tmpdir)
    out = np.empty((NPAD, D), np.float32)
    for c in range(NCORES):
        lo = c * NOWN
        res_t = res.results[c]["out"].T            # [NOWN, D], slot order
        perm_rows = (orders[c][:, None] * P + np.arange(P)).ravel()
        out[lo + perm_rows] = res_t
    return out[:N], res


def kernel(**inputs):
    out, _ = run(**inputs)
    return out
